# revision 46
# baseline (speedup 1.0000x reference)
"""DiffPool pooling layer on 8 Trainium2 NeuronCores.

Reference computation (edge_index / batch are unused by the output):
    s      = softmax(x @ Wp + bp, axis=-1)        # [N, C]
    h      = x @ We + be                          # [N, F]
    pooled = s^T @ h                              # [C, F]
    out    = pooled[None] @ Wo + bo               # [1, C, O]

Algebraic restructuring (projection is linear):
    pooled = (s^T x) We + colsum(s) be^T
so per node-shard k each core computes the partials
    G_k  = s_k^T x_k            [C, F]
    cs_k = colsum(s_k)          [C]
    out_k = (G_k We + cs_k be^T) Wo + bo/8
and the host sums the eight [C, O] partials (the unshard step).
No h materialization, no collectives.

Layout: nodes are block-assigned to partitions (partition p holds nodes
p*48..p*48+47 of the first 6144; the 106-node tail is node-major). This
makes the x DMA 16KB-contiguous per partition line (descriptor-cheap).
Any node->partition assignment is valid because the G contraction only
requires s and x to agree on it.

Per 128-node tile j (x resident in SBUF as fp16, cast during SWDGE DMA):
  - PE transposes 4 f-chunks -> xT (fp16 PSUM) -> DVE copy to SBUF
  - logits = ones x bp + sum_k xT_k^T @ Wp_k    (fp16 MMs, fp32 PSUM)
  - ACT exp w/ accum_out -> unnormalized s + row sums; DVE recip+scale
  - G/cs matmuls are software-pipelined several tiles behind so the PE
    does not stall on the softmax chain.
Final (once per core): project the partial in fp32 on PE.
Measured ~66-70us per core-pass on HW (8 cores in parallel); the x load
(12.8MB fp32 -> fp16 cast-DMA) is ~43us of that and overlaps compute.
"""

import numpy as np
from contextlib import ExitStack

N_ALL, F, C, O = 50000, 512, 64, 256
NCORES = 8
NLOC = N_ALL // NCORES          # 6250 nodes per core
P = 128
KC = F // P                     # 4 feature chunks
JROWS = 48                      # node tiles in the main block
NMAIN = P * JROWS               # 6144 nodes in the main block
NTAIL = NLOC - NMAIN            # 106-node tail
NSPLIT = 12                     # main-block DMA slices (4 tiles each)
JS = JROWS // NSPLIT            # tiles per slice

_CACHE = {}


def _main_loop(nc, mybir, x_d, xs_parts, x_tail, xs32pool, x32_tail,
               xtpool, spool, smallp, pxt, plg,
               ident16, ones_row16, ones_col16, wp_h, bp_h, g_ps, cs_ps,
               parts="full32"):
    """One full pass over this core's node shard, accumulating G / colsum."""
    f32 = mybir.dt.float32
    f16 = mybir.dt.float16
    AF = mybir.ActivationFunctionType

    if parts == "nd_empty":
        # For_i back-edge floor: a trivial body
        zt = smallp.tile([P, 1], f32, tag="zz", name="zz")
        nc.vector.memset(zt[:], 0.0)
        return

    # x DMAs: main block as NSPLIT slices, 16KB contiguous per partition
    xm = x_d[0:NMAIN, :].rearrange("(p j) f -> p j f", p=P)
    if parts == "dma32":
        # ablation: HWDGE fp32 loads (no cast) into fp32 scratch
        for i in range(NSPLIT):
            nc.sync.dma_start(xs_parts[i][:], xm[:, i * JS : (i + 1) * JS, :])
        nc.sync.dma_start(x_tail[0:NTAIL, :], x_d[NMAIN:NLOC, :])
        return
    if parts == "dma32b":
        # ablation: HWDGE fp32 loads split across both HWDGE rings
        for i in range(NSPLIT):
            eng = nc.sync if i % 2 == 0 else nc.scalar
            eng.dma_start(xs_parts[i][:], xm[:, i * JS : (i + 1) * JS, :])
        nc.scalar.dma_start(x_tail[0:NTAIL, :], x_d[NMAIN:NLOC, :])
        return
    if parts in ("full32", "cast32", "v3", "v3c", "v4", "v4c"):
        # v2/v3 load: HWDGE fp32 into rotating staging + on-chip cast to fp16
        def cast_dve(dst, src):
            nc.vector.tensor_copy(dst, src)

        def cast_act(dst, src):
            nc.scalar.activation(dst, src, AF.Copy)

        def cast_gps(dst, src):
            nc.gpsimd.tensor_copy(dst, src)

        if parts in ("full32", "cast32"):
            cast_engs = [cast_gps]
        elif parts in ("v4", "v4c"):
            # spread the cast across the three underused engines
            cast_engs = [cast_dve, cast_act, cast_gps]
        else:
            cast_engs = [cast_dve]
        for i in range(NSPLIT):
            t32 = xs32pool.tile([P, JS, F], f32, tag="xs32", name=f"xs32_{i}")
            nc.sync.dma_start(t32[:], xm[:, i * JS : (i + 1) * JS, :])
            cast_engs[i % len(cast_engs)](xs_parts[i][:], t32[:])
        nc.sync.dma_start(x32_tail[0:NTAIL, :], x_d[NMAIN:NLOC, :])
        cast_engs[0](x_tail[0:NTAIL, :], x32_tail[0:NTAIL, :])
        if parts in ("cast32", "v3c", "v4c"):
            return
    elif parts in ("full", "dma"):
        nc.gpsimd.dma_start(x_tail[0:NTAIL, :], x_d[NMAIN:NLOC, :])
        for i in range(NSPLIT):
            nc.gpsimd.dma_start(xs_parts[i][:], xm[:, i * JS : (i + 1) * JS, :])
    # nodma / nd_* variants: no x load at all (x memset in _build)

    if parts == "dma":
        return

    # tile list: (x view full-partition, active rows)
    tiles = [(xs_parts[j // JS][:, j % JS, :], P) for j in range(JROWS)]
    tiles.append((x_tail[:, :], NTAIL))
    ntiles = len(tiles)

    if parts in ("v4", "v4n"):
        # pair-batched pipeline: 2 tiles per PSUM tile / DVE op
        groups = [(2 * g, min(2, ntiles - 2 * g))
                  for g in range((ntiles + 1) // 2)]
        ngroups = len(groups)
        xt_sbs2 = {}
        s_pairs = {}

        def p_transp(g):
            j0, gn = groups[g]
            xt_ps = pxt.tile([P, 2, KC, P], f16, tag="xt_ps", name="xt_ps")
            for t in range(gn):
                xv, nt = tiles[j0 + t]
                for k in range(KC):
                    nc.tensor.transpose(
                        xt_ps[:, t, k, 0:nt],
                        xv[0:nt, k * P : (k + 1) * P],
                        ident16[0:nt, 0:nt],
                    )
            xt_sb = xtpool.tile([P, 2, KC, P], f16, tag="xt_sb", name="xt_sb")
            nt_last = tiles[j0 + gn - 1][1]
            if gn == 2 and nt_last == P:
                nc.vector.tensor_copy(
                    xt_sb[:].bitcast(f32), xt_ps[:].bitcast(f32)
                )
            else:
                nc.vector.tensor_copy(
                    xt_sb[:, 0:gn, :, 0:nt_last].bitcast(f32),
                    xt_ps[:, 0:gn, :, 0:nt_last].bitcast(f32),
                )
            xt_sbs2[g] = xt_sb

        def p_logits(g):
            j0, gn = groups[g]
            xt_sb = xt_sbs2.pop(g)
            lg_ps = plg.tile([P, 2, C], f32, tag="lg_ps", name="lg_ps")
            for t in range(gn):
                nt = tiles[j0 + t][1]
                nc.tensor.matmul(
                    lg_ps[0:nt, t, :], ones_row16[:, 0:nt], bp_h[:],
                    start=True, stop=False,
                )
                for k in range(KC):
                    nc.tensor.matmul(
                        lg_ps[0:nt, t, :], xt_sb[:, t, k, 0:nt], wp_h[:, k, :],
                        start=False, stop=(k == KC - 1),
                    )
            return lg_ps

        def p_softmax(g, lg_ps):
            j0, gn = groups[g]
            nt_last = tiles[j0 + gn - 1][1]
            se = spool.tile([P, 2, C], f32, tag="se", name="se")
            rs = smallp.tile([P, 2], f32, tag="rs", name="rs")
            for t in range(gn):
                nt = tiles[j0 + t][1]
                nc.scalar.activation(
                    se[0:nt, t, :], lg_ps[0:nt, t, :], AF.Exp,
                    accum_out=rs[0:nt, t : t + 1],
                )
            ri = smallp.tile([P, 2], f32, tag="ri", name="ri")
            if gn == 2 and nt_last == P:
                nc.vector.reciprocal(ri[:, :], rs[:, :])
            else:
                nc.vector.reciprocal(ri[0:nt_last, 0:gn], rs[0:nt_last, 0:gn])
            s_h = spool.tile([P, 2, C], f16, tag="s_h", name="s_h")
            for t in range(gn):
                nt = tiles[j0 + t][1]
                nc.vector.tensor_scalar_mul(
                    s_h[0:nt, t, :], se[0:nt, t, :], ri[0:nt, t : t + 1]
                )
            s_pairs[g] = s_h

        def p_gcs(g, last):
            j0, gn = groups[g]
            s_h = s_pairs.pop(g)
            for t in range(gn):
                xv, nt = tiles[j0 + t]
                is_last = last and t == gn - 1
                nc.tensor.matmul(
                    g_ps[:], s_h[0:nt, t, 0:C], xv[0:nt, :],
                    start=(j0 + t == 0), stop=is_last,
                )
                nc.tensor.matmul(
                    cs_ps[:], s_h[0:nt, t, 0:C], ones_col16[0:nt, :],
                    start=(j0 + t == 0), stop=is_last,
                )

        GSKEW = 2
        p_transp(0)
        for g in range(ngroups):
            if g + 1 < ngroups:
                p_transp(g + 1)
            lg_ps = p_logits(g)
            if g >= GSKEW:
                p_gcs(g - GSKEW, last=False)
            p_softmax(g, lg_ps)
        for g in range(ngroups - GSKEW, ngroups):
            p_gcs(g, last=(g == ngroups - 1))
        return

    # software pipeline so PE never waits on DVE/ACT:
    # at step j, PE runs: transp(j+1) | logits(j) | G/cs(j-SKEW)
    xt_sbs = {}   # j -> xt_sb
    s_views = {}  # j -> s view for G/cs

    nd_g_family = ("nd_g", "nd_gonly", "nd_csonly", "nd_gcs1", "nd_gbatch",
                   "nd_trg")
    cs_col = parts in ("nd_gcs1", "nd_gbatch", "v3", "v3n")
    cs_batch = parts == "nd_gbatch"
    saved_s = []

    def emit_transp(j):
        if (parts in nd_g_family and parts != "nd_trg") or parts == "nd_lgexp":
            return
        xv, nt = tiles[j]
        if parts == "nd_trg":
            nt = P  # tail rows are memset; transpose full width
        xt_ps = pxt.tile([P, KC, P], f16, tag="xt_ps", name="xt_ps")
        for k in range(KC):
            nc.tensor.transpose(
                xt_ps[:, k, 0:nt],
                xv[0:nt, k * P : (k + 1) * P],
                ident16[0:nt, 0:nt],
            )
        xt_sb = xtpool.tile([P, KC, P], f16, tag="xt_sb", name="xt_sb")
        # fp16 pairs copied as fp32 halves the DVE element count
        nc.vector.tensor_copy(
            xt_sb[:, :, 0:nt].bitcast(f32), xt_ps[:, :, 0:nt].bitcast(f32)
        )
        xt_sbs[j] = xt_sb

    def emit_logits(j):
        _, nt = tiles[j]
        if parts in nd_g_family:
            return None
        if parts == "nd_lgexp":
            xt_sb = None
        else:
            xt_sb = xt_sbs.pop(j)
        lg_ps = plg.tile([P, C], f32, tag="lg_ps", name="lg_ps")
        nc.tensor.matmul(
            lg_ps[0:nt, :], ones_row16[:, 0:nt], bp_h[:],
            start=True, stop=False,
        )
        for k in range(KC):
            lhs = ident16[:, 0:nt] if xt_sb is None else xt_sb[:, k, 0:nt]
            nc.tensor.matmul(
                lg_ps[0:nt, :], lhs, wp_h[:, k, :],
                start=False, stop=(k == KC - 1),
            )
        return lg_ps

    def emit_softmax(j, lg_ps):
        _, nt = tiles[j]
        if parts in nd_g_family:
            s_views[j] = ident16[0:nt, 0:C]
            return
        if parts in ("nd_exp", "nd_lgexp"):
            # unnormalized exp straight to fp16 (timing ablation)
            s_h = spool.tile([P, C], f16, tag="s_h", name="s_h")
            rs = smallp.tile([P, 1], f32, tag="rs", name="rs")
            nc.scalar.activation(
                s_h[0:nt, :], lg_ps[0:nt, :], AF.Exp, accum_out=rs[0:nt, :]
            )
            s_views[j] = s_h[0:nt, :]
            return
        se = spool.tile([P, C], f32, tag="se", name="se")
        rs = smallp.tile([P, 1], f32, tag="rs", name="rs")
        nc.scalar.activation(
            se[0:nt, :], lg_ps[0:nt, :], AF.Exp, accum_out=rs[0:nt, :]
        )
        ri = smallp.tile([P, 1], f32, tag="ri", name="ri")
        nc.vector.reciprocal(ri[0:nt, :], rs[0:nt, :])
        s_h = spool.tile([P, C], f16, tag="s_h", name="s_h")
        if parts in ("v3", "v3n"):
            # normalize on ACT (per-partition scale), keeping DVE light
            nc.scalar.activation(
                s_h[0:nt, :], se[0:nt, :], AF.Copy, scale=ri[0:nt, :]
            )
        else:
            nc.vector.tensor_scalar_mul(s_h[0:nt, :], se[0:nt, :], ri[0:nt, :])
        s_views[j] = s_h[0:nt, :]

    def emit_gcs(j, last):
        xv, nt = tiles[j]
        s_view = s_views.pop(j)
        if parts == "nd_trg":
            xt_sb = xt_sbs.pop(j)
            nc.tensor.matmul(
                g_ps[:], ident16[:, 0:C], xt_sb[:, :, :],
                start=(j == 0), stop=last,
            )
            return
        if parts != "nd_csonly":
            nc.tensor.matmul(
                g_ps[:], s_view, xv[0:nt, :],
                start=(j == 0), stop=last,
            )
        if parts in ("nd_gonly",):
            return
        if cs_batch:
            saved_s.append((s_view, nt))
        elif cs_col:
            # cs^T [C,1]: reuse s as the stationary (no weight reload)
            nc.tensor.matmul(
                cs_ps[:], s_view, ones_col16[0:nt, :],
                start=(j == 0), stop=last,
            )
        else:
            nc.tensor.matmul(
                cs_ps[:], ones_col16[0:nt, :], s_view,
                start=(j == 0), stop=last,
            )

    SKEW = 4
    emit_transp(0)
    for j in range(ntiles):
        if j + 1 < ntiles:
            emit_transp(j + 1)
        lg_ps = emit_logits(j)
        if j >= SKEW:
            emit_gcs(j - SKEW, last=False)
        emit_softmax(j, lg_ps)
    for j in range(ntiles - SKEW, ntiles):
        emit_gcs(j, last=(j == ntiles - 1))
    if cs_batch:
        for idx, (sv, nt) in enumerate(saved_s):
            nc.tensor.matmul(
                cs_ps[:], sv, ones_col16[0:nt, :],
                start=(idx == 0), stop=(idx == len(saved_s) - 1),
            )


def _build(bench_reps=None, parts="v4"):
    """Build the bass module. bench_reps: if set, wrap the main node loop
    in a hardware For_i repeating it that many times (timing-only variant:
    x and weights live on device, no input transfer)."""
    import concourse.mybir as mybir
    import concourse.tile as tile
    from concourse import bacc
    from concourse.masks import make_identity

    f32 = mybir.dt.float32
    f16 = mybir.dt.float16

    nc = bacc.Bacc(
        "TRN2", target_bir_lowering=False, debug=False, num_devices=NCORES
    )

    if bench_reps:
        x_d = nc.dram_tensor("xint", [NLOC, F], f32, kind="Internal").ap()
        wp_d = bp_d = we_d = be_d = wo_d = bo_d = None
    else:
        x_d = nc.dram_tensor("x", [NLOC, F], f32, kind="ExternalInput").ap()
        wp_d = nc.dram_tensor("wp", [F, C], f32, kind="ExternalInput").ap()
        bp_d = nc.dram_tensor("bp", [1, C], f32, kind="ExternalInput").ap()
        we_d = nc.dram_tensor("we", [F, F], f32, kind="ExternalInput").ap()
        be_d = nc.dram_tensor("be", [1, F], f32, kind="ExternalInput").ap()
        wo_d = nc.dram_tensor("wo", [F, O], f32, kind="ExternalInput").ap()
        bo_d = nc.dram_tensor("bo8", [1, O], f32, kind="ExternalInput").ap()
    out_d = nc.dram_tensor("out", [C, O], f32, kind="ExternalOutput").ap()

    with tile.TileContext(nc) as tc, ExitStack() as ctx:
        const = ctx.enter_context(tc.tile_pool(name="const", bufs=1))
        accp = ctx.enter_context(tc.tile_pool(name="accp", bufs=1, space="PSUM"))

        ident16 = const.tile([P, P], f16)
        make_identity(nc, ident16[:])
        ident32 = const.tile([C, C], f32)
        make_identity(nc, ident32[:])
        ones_row16 = const.tile([1, P], f16)
        nc.vector.memset(ones_row16[:], 1.0)
        ones_col16 = const.tile([P, 1], f16)
        nc.vector.memset(ones_col16[:], 1.0)
        ones_row32 = const.tile([1, P], f32)
        nc.vector.memset(ones_row32[:], 1.0)

        # resident x (fp16): NSPLIT main slices + node-major tail
        xdt = f32 if parts in ("dma32", "dma32b") else f16
        xs_parts = [
            const.tile([P, JS, F], xdt, name=f"xs{i}") for i in range(NSPLIT)
        ]
        x_tail = const.tile([P, F], xdt, name="x_tail")
        x32_tail = (
            const.tile([P, F], f32, name="x32_tail")
            if parts in ("full32", "cast32", "v3", "v3c", "v4", "v4c")
            else None
        )

        # weights: [F, M] -> [128, KC, M] (partition = f within chunk)
        wp_sb = const.tile([P, KC, C], f32)
        wp_h = const.tile([P, KC, C], f16)
        bp_h = const.tile([1, C], f16)
        we_sb = const.tile([P, KC, F], f32)
        be_sb = const.tile([1, F], f32)
        wo_sb = const.tile([P, KC, O], f32)
        bo_sb = const.tile([1, O], f32)
        if bench_reps:
            for tl in (wp_sb, bp_h, we_sb, be_sb, wo_sb, bo_sb):
                nc.vector.memset(tl[:], 0.0)
        else:
            nc.gpsimd.dma_start(
                wp_sb[:], wp_d.rearrange("(kc p) c -> p kc c", p=P)
            )
            nc.gpsimd.dma_start(bp_h[:], bp_d)  # cast during DMA
            nc.gpsimd.dma_start(
                we_sb[:], we_d.rearrange("(kc p) f -> p kc f", p=P)
            )
            nc.gpsimd.dma_start(be_sb[:], be_d)
            nc.gpsimd.dma_start(
                wo_sb[:], wo_d.rearrange("(kc p) o -> p kc o", p=P)
            )
            nc.gpsimd.dma_start(bo_sb[:], bo_d)
        nc.gpsimd.tensor_copy(wp_h[:], wp_sb[:])

        # persistent accumulators (one PSUM bank each)
        cs_col_mode = parts in ("nd_gcs1", "nd_gbatch", "v3", "v3n",
                                "v4", "v4n")
        g_ps = accp.tile([C, F], f32)
        cs_ps = accp.tile([C, 1] if cs_col_mode else [1, C], f32)
        if parts in ("nd_gonly", "nd_trg"):
            nc.vector.memset(cs_ps[:], 0.0)
        if parts == "nd_csonly":
            nc.vector.memset(g_ps[:], 0.0)

        if bench_reps:
            # zero-fill internal x so the compute sees finite data
            zt = const.tile([P, JS, F], f32, name="zt")
            nc.vector.memset(zt[:], 0.0)
            xm = x_d[0:NMAIN, :].rearrange("(p j) f -> p j f", p=P)
            for i in range(NSPLIT):
                nc.sync.dma_start(xm[:, i * JS : (i + 1) * JS, :], zt[:])
            nc.sync.dma_start(x_d[NMAIN:NLOC, :], zt[0:NTAIL, 0, :])
            if parts in ("nodma", "v3n", "v4n") or parts.startswith("nd_"):
                for i in range(NSPLIT):
                    nc.vector.memset(xs_parts[i][:], 0.0)
                nc.vector.memset(x_tail[:], 0.0)

        with ExitStack() as lctx:
            xtpool = lctx.enter_context(tc.tile_pool(name="xtpool", bufs=4))
            spool = lctx.enter_context(tc.tile_pool(name="spool", bufs=8))
            smallp = lctx.enter_context(tc.tile_pool(name="smallp", bufs=8))
            xs32pool = lctx.enter_context(tc.tile_pool(name="xs32p", bufs=4))
            pxt = lctx.enter_context(
                tc.tile_pool(name="pxt", bufs=3, space="PSUM")
            )
            plg = lctx.enter_context(
                tc.tile_pool(name="plg", bufs=3, space="PSUM")
            )

            rep_ctx = (
                tc.For_i(0, bench_reps, 1) if bench_reps else ExitStack()
            )
            with rep_ctx:
                _main_loop(
                    nc, mybir, x_d, xs_parts, x_tail, xs32pool, x32_tail,
                    xtpool, spool, smallp, pxt, plg,
                    ident16, ones_row16, ones_col16, wp_h, bp_h,
                    g_ps, cs_ps, parts=parts,
                )

        if parts in ("dma", "dma32", "dma32b", "cast32", "v3c", "v4c",
                     "nd_empty"):
            with ExitStack() as fctx:
                fin0 = fctx.enter_context(tc.tile_pool(name="fin0", bufs=1))
                dummy = fin0.tile([C, O], f32, name="dummy")
                nc.vector.memset(dummy[:], 0.0)
                nc.sync.dma_start(out_d, dummy[:])
        elif True:
            # ---- final projection of the per-core partial (fp32) ----
            with ExitStack() as fctx:
                fin = fctx.enter_context(tc.tile_pool(name="fin", bufs=1))
                pfin = fctx.enter_context(
                    tc.tile_pool(name="pfin", bufs=1, space="PSUM")
                )

                g_sb = fin.tile([C, F], f32)
                nc.vector.tensor_copy(g_sb[:], g_ps[:])
                if cs_col_mode:
                    csc_sb = fin.tile([C, 1], f32)
                    nc.vector.tensor_copy(csc_sb[:], cs_ps[:])
                    csT_ps = pfin.tile([1, C], f32, name="csT_ps")
                    nc.tensor.transpose(
                        csT_ps[:], csc_sb[:], ident32[0:C, 0:C]
                    )
                    cs_sb = fin.tile([1, C], f32)
                    nc.vector.tensor_copy(cs_sb[:], csT_ps[:])
                else:
                    cs_sb = fin.tile([1, C], f32)
                    nc.vector.tensor_copy(cs_sb[:], cs_ps[:])

                # G^T chunks [128, C] so fin lands on partitions
                gt_ps = pfin.tile([P, KC, C], f32, name="gt_ps")
                for k in range(KC):
                    nc.tensor.transpose(
                        gt_ps[:, k, :], g_sb[:, k * P : (k + 1) * P], ident32[:]
                    )
                gt_sb = fin.tile([P, KC, C], f32)
                nc.vector.tensor_copy(gt_sb[:], gt_ps[:])

                # pooledT[fo, c] = sum_fin We[fin, fo] G^T[fin, c] + be[fo] cs[c]
                pt_ps = pfin.tile([P, KC, C], f32, name="pt_ps")
                for j in range(KC):
                    nc.tensor.matmul(
                        pt_ps[:, j, :],
                        be_sb[:, j * P : (j + 1) * P],
                        cs_sb[:],
                        start=True,
                        stop=False,
                    )
                    for k in range(KC):
                        nc.tensor.matmul(
                            pt_ps[:, j, :],
                            we_sb[:, k, j * P : (j + 1) * P],
                            gt_sb[:, k, :],
                            start=False,
                            stop=(k == KC - 1),
                        )
                pt_sb = fin.tile([P, KC, C], f32)
                nc.vector.tensor_copy(pt_sb[:], pt_ps[:])

                # out[c, o] = sum_fo pooledT[fo, c] Wo[fo, o] + bo/8
                out_ps = pfin.tile([C, O], f32, name="out_ps")
                nc.tensor.matmul(
                    out_ps[:], ones_row32[:, 0:C], bo_sb[:],
                    start=True, stop=False,
                )
                for j in range(KC):
                    nc.tensor.matmul(
                        out_ps[:], pt_sb[:, j, :], wo_sb[:, j, :],
                        start=False, stop=(j == KC - 1),
                    )
                out_sb = fin.tile([C, O], f32)
                nc.vector.tensor_copy(out_sb[:], out_ps[:])
                nc.sync.dma_start(out_d, out_sb[:])

    nc.compile()
    return nc


def _get_nc(bench_reps=None, parts="v4"):
    key = ("nc", bench_reps, parts)
    if key not in _CACHE:
        _CACHE[key] = _build(bench_reps, parts)
    return _CACHE[key]


def kernel(x, edge_index=None, batch=None, Wp=None, bp=None, We=None,
           be=None, Wo=None, bo=None, **_unused):
    from concourse.bass_utils import run_bass_kernel_spmd

    x = np.ascontiguousarray(np.asarray(x, dtype=np.float32))
    Wp = np.ascontiguousarray(np.asarray(Wp, dtype=np.float32))
    bp = np.ascontiguousarray(np.asarray(bp, dtype=np.float32)).reshape(1, C)
    We = np.ascontiguousarray(np.asarray(We, dtype=np.float32))
    be = np.ascontiguousarray(np.asarray(be, dtype=np.float32)).reshape(1, F)
    Wo = np.ascontiguousarray(np.asarray(Wo, dtype=np.float32))
    bo8 = np.ascontiguousarray(
        np.asarray(bo, dtype=np.float32).reshape(1, O) / np.float32(NCORES)
    )

    nc = _get_nc()
    in_maps = []
    for k in range(NCORES):
        in_maps.append(
            {
                "x": np.ascontiguousarray(x[k * NLOC : (k + 1) * NLOC]),
                "wp": Wp,
                "bp": bp,
                "we": We,
                "be": be,
                "wo": Wo,
                "bo8": bo8,
            }
        )
    res = run_bass_kernel_spmd(nc, in_maps, core_ids=list(range(NCORES)))
    out = np.zeros((C, O), np.float32)
    for r in res.results:
        out = out + r["out"]
    return out[None]  # [1, C, O]



# revision 68
# speedup vs baseline: 1.1684x; 1.1684x over previous
"""DiffPool pooling layer on 8 Trainium2 NeuronCores.

Reference computation (edge_index / batch are unused by the output):
    s      = softmax(x @ Wp + bp, axis=-1)        # [N, C]
    h      = x @ We + be                          # [N, F]
    pooled = s^T @ h                              # [C, F]
    out    = pooled[None] @ Wo + bo               # [1, C, O]

Algebraic restructuring (projection is linear):
    pooled = (s^T x) We + colsum(s) be^T
so per node-shard k each core computes the partials
    G_k  = s_k^T x_k            [C, F]
    cs_k = colsum(s_k)          [C]
    out_k = (G_k We + cs_k be^T) Wo + bo/8
and the host sums the eight [C, O] partials (the unshard step).
No h materialization, no collectives.

Layout: nodes are block-assigned to partitions (partition p holds nodes
p*48..p*48+47 of the first 6144; the 106-node tail is node-major). This
makes the x DMA 8KB-contiguous per partition line (descriptor-cheap).
Any node->partition assignment is valid because the G contraction only
requires s and x to agree on it.

x load (parts="v12", the production path): 12 HWDGE fp32 slice DMAs on the
sync ring (~325 GB/s, near the 358 GB/s/core HBM cap; SWDGE cast-DMA only
manages ~170 GB/s and also poisons the shared SDMA engines, so it is not
used). fp32 slices land in a rotating staging pool; fp32->fp16 casts are
emitted just-in-time inside the compute loop, alternating DVE/ACT, so a
cast waiting on its DMA never head-of-line-blocks an engine FIFO in front
of ready pipeline work.

Compute, per group of B=4 128-node tiles (batched to amortize per-op overhead):
  - PE: 8 transposes -> xT pair (fp16, one PSUM bank)
  - DVE: one bitcast-fp32 copy of the pair -> SBUF
  - PE: 10 logits MMs (bias + 4 f-chunks per tile, fp16, fp32 PSUM)
  - ACT: exp per tile w/ accum_out row sums; DVE: one pair reciprocal +
    per-tile scale -> s (fp16)
  - PE: per tile G += s^T x  [C,512] and cs += s^T 1  [C,1]. cs uses s as
    the stationary (shared with G) and its own PSUM bank: interleaving two
    accumulation groups with ALTERNATING stationaries (the old
    cs = ones^T s form) costs ~1.1us/tile extra on HW; the shared-
    stationary [C,1] form is ~150ns. The [C,1] colsum is transposed to
    [1,C] once, in the projection.
  - G/cs run GSKEW=2 pair-groups behind the softmax chain.
Final (once per core): project the partial in fp32 on PE.
Measured ~61-63us per core-pass on HW vs ~71-73us for the previous
SWDGE-cast-DMA baseline in the same process (device-speed drift between
processes is +-20-30%, so same-process comparison is the only reliable
one).
"""

import numpy as np
from contextlib import ExitStack

N_ALL, F, C, O = 50000, 512, 64, 256
NCORES = 8
NLOC = N_ALL // NCORES          # 6250 nodes per core
P = 128
KC = F // P                     # 4 feature chunks
JROWS = 48                      # node tiles in the main block
NMAIN = P * JROWS               # 6144 nodes in the main block
NTAIL = NLOC - NMAIN            # 106-node tail
NSPLIT = 12                     # main-block DMA slices (4 tiles each)
JS = JROWS // NSPLIT            # tiles per slice

_CACHE = {}


def _main_loop(nc, mybir, x_d, xs_parts, x_tail, xs32pool, x32_tail,
               xtpool, spool, smallp, pxt, plg,
               ident16, ones_row16, ones_col16, wp_h, bp_h, g_ps, cs_ps,
               parts="full32"):
    """One full pass over this core's node shard, accumulating G / colsum."""
    f32 = mybir.dt.float32
    f16 = mybir.dt.float16
    AF = mybir.ActivationFunctionType

    if parts == "nd_empty":
        # For_i back-edge floor: a trivial body
        zt = smallp.tile([P, 1], f32, tag="zz", name="zz")
        nc.vector.memset(zt[:], 0.0)
        return

    pending_casts = []  # (engine, dst, src, rows) deferred into the loop

    # x DMAs: main block as NSPLIT slices, 16KB contiguous per partition
    xm = x_d[0:NMAIN, :].rearrange("(p j) f -> p j f", p=P)
    if parts == "dma32":
        # ablation: HWDGE fp32 loads (no cast) into fp32 scratch
        for i in range(NSPLIT):
            nc.sync.dma_start(xs_parts[i][:], xm[:, i * JS : (i + 1) * JS, :])
        nc.sync.dma_start(x_tail[0:NTAIL, :], x_d[NMAIN:NLOC, :])
        return
    if parts == "dma32b":
        # ablation: HWDGE fp32 loads split across both HWDGE rings
        for i in range(NSPLIT):
            eng = nc.sync if i % 2 == 0 else nc.scalar
            eng.dma_start(xs_parts[i][:], xm[:, i * JS : (i + 1) * JS, :])
        nc.scalar.dma_start(x_tail[0:NTAIL, :], x_d[NMAIN:NLOC, :])
        return
    if parts in ("full32", "cast32", "v3", "v3c", "v4", "v4c"):
        # v2/v3 load: HWDGE fp32 into rotating staging + on-chip cast to fp16
        def cast_dve(dst, src):
            nc.vector.tensor_copy(dst, src)

        def cast_act(dst, src):
            nc.scalar.activation(dst, src, AF.Copy)

        def cast_gps(dst, src):
            nc.gpsimd.tensor_copy(dst, src)

        if parts in ("full32", "cast32"):
            cast_engs = [cast_gps]
        elif parts in ("v4", "v4c"):
            # spread the cast across the three underused engines
            cast_engs = [cast_dve, cast_act, cast_gps]
        else:
            cast_engs = [cast_dve]
        for i in range(NSPLIT):
            t32 = xs32pool.tile([P, JS, F], f32, tag="xs32", name=f"xs32_{i}")
            nc.sync.dma_start(t32[:], xm[:, i * JS : (i + 1) * JS, :])
            cast_engs[i % len(cast_engs)](xs_parts[i][:], t32[:])
        nc.sync.dma_start(x32_tail[0:NTAIL, :], x_d[NMAIN:NLOC, :])
        cast_engs[0](x_tail[0:NTAIL, :], x32_tail[0:NTAIL, :])
        if parts in ("cast32", "v3c", "v4c"):
            return
    elif parts in ("v5", "v5c"):
        # HWDGE fp32 (+DVE/ACT cast) for the first NHW slices, concurrent
        # SWDGE cast-DMA (hardware fp32->fp16, own queue) for the rest
        NHW = 8
        for i in range(NHW):
            t32 = xs32pool.tile([P, JS, F], f32, tag="xs32", name=f"xs32_{i}")
            nc.sync.dma_start(t32[:], xm[:, i * JS : (i + 1) * JS, :])
            if i % 2 == 0:
                nc.vector.tensor_copy(xs_parts[i][:], t32[:])
            else:
                nc.scalar.activation(xs_parts[i][:], t32[:], AF.Copy)
        for i in range(NHW, NSPLIT):
            nc.gpsimd.dma_start(xs_parts[i][:], xm[:, i * JS : (i + 1) * JS, :])
        nc.gpsimd.dma_start(x_tail[0:NTAIL, :], x_d[NMAIN:NLOC, :])
        if parts == "v5c":
            return
    elif parts in ("v8", "v8d", "v12"):
        # HWDGE fp32 for everything; DVE/ACT casts are emitted just-in-time
        # inside the compute loop so they never head-of-line-block the
        # engines' FIFO queues
        for i in range(NSPLIT):
            t32 = xs32pool.tile([P, JS, F], f32, tag="xs32", name=f"xs32_{i}")
            nc.sync.dma_start(t32[:], xm[:, i * JS : (i + 1) * JS, :])
            eng = "dve" if (parts == "v8d" or i % 2 == 0) else "act"  # v8/v12 alternate
            pending_casts.append((eng, xs_parts[i], t32, None))
        nc.sync.dma_start(x32_tail[0:NTAIL, :], x_d[NMAIN:NLOC, :])
        pending_casts.append(("dve", x_tail, x32_tail, NTAIL))
    elif parts in ("v6", "v6c"):
        # DVE/ACT stay out of the load path entirely: SWDGE cast-DMA for the
        # last slices + tail (own queue, hardware cast), HWDGE fp32 + gpsimd
        # cast for the first NHW6 slices. Both DMA paths run concurrently.
        NHW6 = 8
        for i in range(NHW6, NSPLIT):
            nc.gpsimd.dma_start(xs_parts[i][:], xm[:, i * JS : (i + 1) * JS, :])
        nc.gpsimd.dma_start(x_tail[0:NTAIL, :], x_d[NMAIN:NLOC, :])
        for i in range(NHW6):
            t32 = xs32pool.tile([P, JS, F], f32, tag="xs32", name=f"xs32_{i}")
            nc.sync.dma_start(t32[:], xm[:, i * JS : (i + 1) * JS, :])
            nc.gpsimd.tensor_copy(xs_parts[i][:], t32[:])
        if parts == "v6c":
            return
    elif parts in ("full", "dma"):
        nc.gpsimd.dma_start(x_tail[0:NTAIL, :], x_d[NMAIN:NLOC, :])
        for i in range(NSPLIT):
            nc.gpsimd.dma_start(xs_parts[i][:], xm[:, i * JS : (i + 1) * JS, :])
    # nodma / nd_* variants: no x load at all (x memset in _build)

    if parts == "dma":
        return

    # tile list: (x view full-partition, active rows)
    tiles = [(xs_parts[j // JS][:, j % JS, :], P) for j in range(JROWS)]
    tiles.append((x_tail[:, :], NTAIL))
    ntiles = len(tiles)

    if parts in ("v4", "v4n", "v5", "v6", "v8", "v8d", "v12", "v12n"):
        # batch-of-B pipeline: B tiles per PSUM tile / DVE op
        B = 4 if parts in ("v12", "v12n") else 2
        groups = [(B * g, min(B, ntiles - B * g))
                  for g in range((ntiles + B - 1) // B)]
        ngroups = len(groups)
        xt_sbs2 = {}
        s_pairs = {}

        def emit_cast(idx):
            if idx >= len(pending_casts):
                return
            eng, dst, src, rows = pending_casts[idx]
            d = dst[0:rows, :] if rows else dst[:]
            s = src[0:rows, :] if rows else src[:]
            if eng == "dve":
                nc.vector.tensor_copy(d, s)
            else:
                nc.scalar.activation(d, s, AF.Copy)

        def p_transp(g):
            j0, gn = groups[g]
            xt_ps = pxt.tile([P, B, KC, P], f16, tag="xt_ps", name="xt_ps")
            for t in range(gn):
                xv, nt = tiles[j0 + t]
                for k in range(KC):
                    nc.tensor.transpose(
                        xt_ps[:, t, k, 0:nt],
                        xv[0:nt, k * P : (k + 1) * P],
                        ident16[0:nt, 0:nt],
                    )
            xt_sb = xtpool.tile([P, B, KC, P], f16, tag="xt_sb", name="xt_sb")
            nt_last = tiles[j0 + gn - 1][1]
            if gn == B and nt_last == P:
                nc.vector.tensor_copy(
                    xt_sb[:].bitcast(f32), xt_ps[:].bitcast(f32)
                )
            else:
                nc.vector.tensor_copy(
                    xt_sb[:, 0:gn, :, 0:nt_last].bitcast(f32),
                    xt_ps[:, 0:gn, :, 0:nt_last].bitcast(f32),
                )
            xt_sbs2[g] = xt_sb

        def p_logits(g):
            j0, gn = groups[g]
            xt_sb = xt_sbs2.pop(g)
            lg_ps = plg.tile([P, B, C], f32, tag="lg_ps", name="lg_ps")
            for t in range(gn):
                nt = tiles[j0 + t][1]
                nc.tensor.matmul(
                    lg_ps[0:nt, t, :], ones_row16[:, 0:nt], bp_h[:],
                    start=True, stop=False,
                )
                for k in range(KC):
                    nc.tensor.matmul(
                        lg_ps[0:nt, t, :], xt_sb[:, t, k, 0:nt], wp_h[:, k, :],
                        start=False, stop=(k == KC - 1),
                    )
            return lg_ps

        def p_softmax(g, lg_ps):
            j0, gn = groups[g]
            nt_last = tiles[j0 + gn - 1][1]
            se = spool.tile([P, B, C], f32, tag="se", name="se")
            rs = smallp.tile([P, B], f32, tag="rs", name="rs")
            for t in range(gn):
                nt = tiles[j0 + t][1]
                nc.scalar.activation(
                    se[0:nt, t, :], lg_ps[0:nt, t, :], AF.Exp,
                    accum_out=rs[0:nt, t : t + 1],
                )
            ri = smallp.tile([P, B], f32, tag="ri", name="ri")
            if gn == B and nt_last == P:
                nc.vector.reciprocal(ri[:, :], rs[:, :])
            else:
                nc.vector.reciprocal(ri[0:nt_last, 0:gn], rs[0:nt_last, 0:gn])
            s_h = spool.tile([P, B, C], f16, tag="s_h", name="s_h")
            for t in range(gn):
                nt = tiles[j0 + t][1]
                nc.vector.tensor_scalar_mul(
                    s_h[0:nt, t, :], se[0:nt, t, :], ri[0:nt, t : t + 1]
                )
            s_pairs[g] = s_h

        def p_gcs(g, last):
            j0, gn = groups[g]
            s_h = s_pairs.pop(g)
            for t in range(gn):
                xv, nt = tiles[j0 + t]
                is_last = last and t == gn - 1
                nc.tensor.matmul(
                    g_ps[:], s_h[0:nt, t, 0:C], xv[0:nt, :],
                    start=(j0 + t == 0), stop=is_last,
                )
                nc.tensor.matmul(
                    cs_ps[:], s_h[0:nt, t, 0:C], ones_col16[0:nt, :],
                    start=(j0 + t == 0), stop=is_last,
                )

        GSKEW = 2
        next_cast = [0]

        def pace_casts(g):
            # emit casts through (slice consumed by group g) + 1 lookahead
            target = min((B * g) // JS + 2, len(pending_casts))
            while next_cast[0] < target:
                emit_cast(next_cast[0])
                next_cast[0] += 1

        pace_casts(0)
        p_transp(0)
        for g in range(ngroups):
            pace_casts(g + 1)
            if g + 1 < ngroups:
                p_transp(g + 1)
            lg_ps = p_logits(g)
            if g >= GSKEW:
                p_gcs(g - GSKEW, last=False)
            p_softmax(g, lg_ps)
        for g in range(ngroups - GSKEW, ngroups):
            p_gcs(g, last=(g == ngroups - 1))
        return

    # software pipeline so PE never waits on DVE/ACT:
    # at step j, PE runs: transp(j+1) | logits(j) | G/cs(j-SKEW)
    xt_sbs = {}   # j -> xt_sb
    s_views = {}  # j -> s view for G/cs

    nd_g_family = ("nd_g", "nd_gonly", "nd_csonly", "nd_gcs1", "nd_gbatch",
                   "nd_trg")
    cs_col = parts in ("nd_gcs1", "nd_gbatch", "v3", "v3n")
    cs_batch = parts == "nd_gbatch"
    saved_s = []

    def emit_transp(j):
        if (parts in nd_g_family and parts != "nd_trg") or parts == "nd_lgexp":
            return
        xv, nt = tiles[j]
        if parts == "nd_trg":
            nt = P  # tail rows are memset; transpose full width
        xt_ps = pxt.tile([P, KC, P], f16, tag="xt_ps", name="xt_ps")
        for k in range(KC):
            nc.tensor.transpose(
                xt_ps[:, k, 0:nt],
                xv[0:nt, k * P : (k + 1) * P],
                ident16[0:nt, 0:nt],
            )
        xt_sb = xtpool.tile([P, KC, P], f16, tag="xt_sb", name="xt_sb")
        # fp16 pairs copied as fp32 halves the DVE element count
        nc.vector.tensor_copy(
            xt_sb[:, :, 0:nt].bitcast(f32), xt_ps[:, :, 0:nt].bitcast(f32)
        )
        xt_sbs[j] = xt_sb

    def emit_logits(j):
        _, nt = tiles[j]
        if parts in nd_g_family:
            return None
        if parts == "nd_lgexp":
            xt_sb = None
        else:
            xt_sb = xt_sbs.pop(j)
        lg_ps = plg.tile([P, C], f32, tag="lg_ps", name="lg_ps")
        nc.tensor.matmul(
            lg_ps[0:nt, :], ones_row16[:, 0:nt], bp_h[:],
            start=True, stop=False,
        )
        for k in range(KC):
            lhs = ident16[:, 0:nt] if xt_sb is None else xt_sb[:, k, 0:nt]
            nc.tensor.matmul(
                lg_ps[0:nt, :], lhs, wp_h[:, k, :],
                start=False, stop=(k == KC - 1),
            )
        return lg_ps

    def emit_softmax(j, lg_ps):
        _, nt = tiles[j]
        if parts in nd_g_family:
            s_views[j] = ident16[0:nt, 0:C]
            return
        if parts in ("nd_exp", "nd_lgexp"):
            # unnormalized exp straight to fp16 (timing ablation)
            s_h = spool.tile([P, C], f16, tag="s_h", name="s_h")
            rs = smallp.tile([P, 1], f32, tag="rs", name="rs")
            nc.scalar.activation(
                s_h[0:nt, :], lg_ps[0:nt, :], AF.Exp, accum_out=rs[0:nt, :]
            )
            s_views[j] = s_h[0:nt, :]
            return
        se = spool.tile([P, C], f32, tag="se", name="se")
        rs = smallp.tile([P, 1], f32, tag="rs", name="rs")
        nc.scalar.activation(
            se[0:nt, :], lg_ps[0:nt, :], AF.Exp, accum_out=rs[0:nt, :]
        )
        ri = smallp.tile([P, 1], f32, tag="ri", name="ri")
        nc.vector.reciprocal(ri[0:nt, :], rs[0:nt, :])
        s_h = spool.tile([P, C], f16, tag="s_h", name="s_h")
        if parts in ("v3", "v3n"):
            # normalize on ACT (per-partition scale), keeping DVE light
            nc.scalar.activation(
                s_h[0:nt, :], se[0:nt, :], AF.Copy, scale=ri[0:nt, :]
            )
        else:
            nc.vector.tensor_scalar_mul(s_h[0:nt, :], se[0:nt, :], ri[0:nt, :])
        s_views[j] = s_h[0:nt, :]

    def emit_gcs(j, last):
        xv, nt = tiles[j]
        s_view = s_views.pop(j)
        if parts == "nd_trg":
            xt_sb = xt_sbs.pop(j)
            nc.tensor.matmul(
                g_ps[:], ident16[:, 0:C], xt_sb[:, :, :],
                start=(j == 0), stop=last,
            )
            return
        if parts != "nd_csonly":
            nc.tensor.matmul(
                g_ps[:], s_view, xv[0:nt, :],
                start=(j == 0), stop=last,
            )
        if parts in ("nd_gonly",):
            return
        if cs_batch:
            saved_s.append((s_view, nt))
        elif cs_col:
            # cs^T [C,1]: reuse s as the stationary (no weight reload)
            nc.tensor.matmul(
                cs_ps[:], s_view, ones_col16[0:nt, :],
                start=(j == 0), stop=last,
            )
        else:
            nc.tensor.matmul(
                cs_ps[:], ones_col16[0:nt, :], s_view,
                start=(j == 0), stop=last,
            )

    SKEW = 4
    emit_transp(0)
    for j in range(ntiles):
        if j + 1 < ntiles:
            emit_transp(j + 1)
        lg_ps = emit_logits(j)
        if j >= SKEW:
            emit_gcs(j - SKEW, last=False)
        emit_softmax(j, lg_ps)
    for j in range(ntiles - SKEW, ntiles):
        emit_gcs(j, last=(j == ntiles - 1))
    if cs_batch:
        for idx, (sv, nt) in enumerate(saved_s):
            nc.tensor.matmul(
                cs_ps[:], sv, ones_col16[0:nt, :],
                start=(idx == 0), stop=(idx == len(saved_s) - 1),
            )


def _build(bench_reps=None, parts="v12"):
    """Build the bass module. bench_reps: if set, wrap the main node loop
    in a hardware For_i repeating it that many times (timing-only variant:
    x and weights live on device, no input transfer)."""
    import concourse.mybir as mybir
    import concourse.tile as tile
    from concourse import bacc
    from concourse.masks import make_identity

    f32 = mybir.dt.float32
    f16 = mybir.dt.float16

    nc = bacc.Bacc(
        "TRN2", target_bir_lowering=False, debug=False, num_devices=NCORES
    )

    if bench_reps:
        x_d = nc.dram_tensor("xint", [NLOC, F], f32, kind="Internal").ap()
        wp_d = bp_d = we_d = be_d = wo_d = bo_d = None
    else:
        x_d = nc.dram_tensor("x", [NLOC, F], f32, kind="ExternalInput").ap()
        wp_d = nc.dram_tensor("wp", [F, C], f32, kind="ExternalInput").ap()
        bp_d = nc.dram_tensor("bp", [1, C], f32, kind="ExternalInput").ap()
        we_d = nc.dram_tensor("we", [F, F], f32, kind="ExternalInput").ap()
        be_d = nc.dram_tensor("be", [1, F], f32, kind="ExternalInput").ap()
        wo_d = nc.dram_tensor("wo", [F, O], f32, kind="ExternalInput").ap()
        bo_d = nc.dram_tensor("bo8", [1, O], f32, kind="ExternalInput").ap()
    out_d = nc.dram_tensor("out", [C, O], f32, kind="ExternalOutput").ap()

    with tile.TileContext(nc) as tc, ExitStack() as ctx:
        const = ctx.enter_context(tc.tile_pool(name="const", bufs=1))
        accp = ctx.enter_context(tc.tile_pool(name="accp", bufs=1, space="PSUM"))

        ident16 = const.tile([P, P], f16)
        make_identity(nc, ident16[:])
        ident32 = const.tile([C, C], f32)
        make_identity(nc, ident32[:])
        ones_row16 = const.tile([1, P], f16)
        nc.vector.memset(ones_row16[:], 1.0)
        ones_col16 = const.tile([P, 1], f16)
        nc.vector.memset(ones_col16[:], 1.0)
        ones_row32 = const.tile([1, P], f32)
        nc.vector.memset(ones_row32[:], 1.0)

        # resident x (fp16): NSPLIT main slices + node-major tail
        xdt = f32 if parts in ("dma32", "dma32b") else f16
        xs_parts = [
            const.tile([P, JS, F], xdt, name=f"xs{i}") for i in range(NSPLIT)
        ]
        x_tail = const.tile([P, F], xdt, name="x_tail")
        x32_tail = (
            const.tile([P, F], f32, name="x32_tail")
            if parts in ("full32", "cast32", "v3", "v3c", "v4", "v4c", "v8", "v8d", "v12")
            else None
        )

        # weights: [F, M] -> [128, KC, M] (partition = f within chunk)
        wp_sb = const.tile([P, KC, C], f32)
        wp_h = const.tile([P, KC, C], f16)
        bp_h = const.tile([1, C], f16)
        we_sb = const.tile([P, KC, F], f32)
        be_sb = const.tile([1, F], f32)
        wo_sb = const.tile([P, KC, O], f32)
        bo_sb = const.tile([1, O], f32)
        if bench_reps:
            for tl in (wp_sb, bp_h, we_sb, be_sb, wo_sb, bo_sb):
                nc.vector.memset(tl[:], 0.0)
        else:
            nc.gpsimd.dma_start(
                wp_sb[:], wp_d.rearrange("(kc p) c -> p kc c", p=P)
            )
            nc.gpsimd.dma_start(bp_h[:], bp_d)  # cast during DMA
            nc.gpsimd.dma_start(
                we_sb[:], we_d.rearrange("(kc p) f -> p kc f", p=P)
            )
            nc.gpsimd.dma_start(be_sb[:], be_d)
            nc.gpsimd.dma_start(
                wo_sb[:], wo_d.rearrange("(kc p) o -> p kc o", p=P)
            )
            nc.gpsimd.dma_start(bo_sb[:], bo_d)
        nc.gpsimd.tensor_copy(wp_h[:], wp_sb[:])

        # persistent accumulators (one PSUM bank each)
        cs_col_mode = parts in ("nd_gcs1", "nd_gbatch", "v3", "v3n",
                                "v4", "v4n", "v5", "v6", "v8", "v8d", "v12", "v12n")
        g_ps = accp.tile([C, F], f32)
        cs_ps = accp.tile([C, 1] if cs_col_mode else [1, C], f32)
        if parts in ("nd_gonly", "nd_trg"):
            nc.vector.memset(cs_ps[:], 0.0)
        if parts == "nd_csonly":
            nc.vector.memset(g_ps[:], 0.0)

        if bench_reps:
            # zero-fill internal x so the compute sees finite data
            zt = const.tile([P, JS, F], f32, name="zt")
            nc.vector.memset(zt[:], 0.0)
            xm = x_d[0:NMAIN, :].rearrange("(p j) f -> p j f", p=P)
            for i in range(NSPLIT):
                nc.sync.dma_start(xm[:, i * JS : (i + 1) * JS, :], zt[:])
            nc.sync.dma_start(x_d[NMAIN:NLOC, :], zt[0:NTAIL, 0, :])
            if parts in ("nodma", "v3n", "v4n", "v12n") or parts.startswith("nd_"):
                for i in range(NSPLIT):
                    nc.vector.memset(xs_parts[i][:], 0.0)
                nc.vector.memset(x_tail[:], 0.0)

        with ExitStack() as lctx:
            xtpool = lctx.enter_context(tc.tile_pool(name="xtpool", bufs=4))
            spool = lctx.enter_context(tc.tile_pool(name="spool", bufs=8))
            smallp = lctx.enter_context(tc.tile_pool(name="smallp", bufs=8))
            xs32pool = lctx.enter_context(tc.tile_pool(name="xs32p", bufs=6))
            psb = 2 if parts in ("v12", "v12n") else 3
            pxt = lctx.enter_context(
                tc.tile_pool(name="pxt", bufs=psb, space="PSUM")
            )
            plg = lctx.enter_context(
                tc.tile_pool(name="plg", bufs=psb, space="PSUM")
            )

            rep_ctx = (
                tc.For_i(0, bench_reps, 1) if bench_reps else ExitStack()
            )
            with rep_ctx:
                _main_loop(
                    nc, mybir, x_d, xs_parts, x_tail, xs32pool, x32_tail,
                    xtpool, spool, smallp, pxt, plg,
                    ident16, ones_row16, ones_col16, wp_h, bp_h,
                    g_ps, cs_ps, parts=parts,
                )

        if parts in ("dma", "dma32", "dma32b", "cast32", "v3c", "v4c",
                     "v5c", "v6c", "nd_empty"):
            with ExitStack() as fctx:
                fin0 = fctx.enter_context(tc.tile_pool(name="fin0", bufs=1))
                dummy = fin0.tile([C, O], f32, name="dummy")
                nc.vector.memset(dummy[:], 0.0)
                nc.sync.dma_start(out_d, dummy[:])
        elif True:
            # ---- final projection of the per-core partial (fp32) ----
            with ExitStack() as fctx:
                fin = fctx.enter_context(tc.tile_pool(name="fin", bufs=1))
                pfin = fctx.enter_context(
                    tc.tile_pool(name="pfin", bufs=1, space="PSUM")
                )

                g_sb = fin.tile([C, F], f32)
                nc.vector.tensor_copy(g_sb[:], g_ps[:])
                if cs_col_mode:
                    csc_sb = fin.tile([C, 1], f32)
                    nc.vector.tensor_copy(csc_sb[:], cs_ps[:])
                    csT_ps = pfin.tile([1, C], f32, name="csT_ps")
                    nc.tensor.transpose(
                        csT_ps[:], csc_sb[:], ident32[0:C, 0:C]
                    )
                    cs_sb = fin.tile([1, C], f32)
                    nc.vector.tensor_copy(cs_sb[:], csT_ps[:])
                else:
                    cs_sb = fin.tile([1, C], f32)
                    nc.vector.tensor_copy(cs_sb[:], cs_ps[:])

                # G^T chunks [128, C] so fin lands on partitions
                gt_ps = pfin.tile([P, KC, C], f32, name="gt_ps")
                for k in range(KC):
                    nc.tensor.transpose(
                        gt_ps[:, k, :], g_sb[:, k * P : (k + 1) * P], ident32[:]
                    )
                gt_sb = fin.tile([P, KC, C], f32)
                nc.vector.tensor_copy(gt_sb[:], gt_ps[:])

                # pooledT[fo, c] = sum_fin We[fin, fo] G^T[fin, c] + be[fo] cs[c]
                pt_ps = pfin.tile([P, KC, C], f32, name="pt_ps")
                for j in range(KC):
                    nc.tensor.matmul(
                        pt_ps[:, j, :],
                        be_sb[:, j * P : (j + 1) * P],
                        cs_sb[:],
                        start=True,
                        stop=False,
                    )
                    for k in range(KC):
                        nc.tensor.matmul(
                            pt_ps[:, j, :],
                            we_sb[:, k, j * P : (j + 1) * P],
                            gt_sb[:, k, :],
                            start=False,
                            stop=(k == KC - 1),
                        )
                pt_sb = fin.tile([P, KC, C], f32)
                nc.vector.tensor_copy(pt_sb[:], pt_ps[:])

                # out[c, o] = sum_fo pooledT[fo, c] Wo[fo, o] + bo/8
                out_ps = pfin.tile([C, O], f32, name="out_ps")
                nc.tensor.matmul(
                    out_ps[:], ones_row32[:, 0:C], bo_sb[:],
                    start=True, stop=False,
                )
                for j in range(KC):
                    nc.tensor.matmul(
                        out_ps[:], pt_sb[:, j, :], wo_sb[:, j, :],
                        start=False, stop=(j == KC - 1),
                    )
                out_sb = fin.tile([C, O], f32)
                nc.vector.tensor_copy(out_sb[:], out_ps[:])
                nc.sync.dma_start(out_d, out_sb[:])

    nc.compile()
    return nc


def _get_nc(bench_reps=None, parts="v12"):
    key = ("nc", bench_reps, parts)
    if key not in _CACHE:
        _CACHE[key] = _build(bench_reps, parts)
    return _CACHE[key]


def kernel(x, edge_index=None, batch=None, Wp=None, bp=None, We=None,
           be=None, Wo=None, bo=None, **_unused):
    from concourse.bass_utils import run_bass_kernel_spmd

    x = np.ascontiguousarray(np.asarray(x, dtype=np.float32))
    Wp = np.ascontiguousarray(np.asarray(Wp, dtype=np.float32))
    bp = np.ascontiguousarray(np.asarray(bp, dtype=np.float32)).reshape(1, C)
    We = np.ascontiguousarray(np.asarray(We, dtype=np.float32))
    be = np.ascontiguousarray(np.asarray(be, dtype=np.float32)).reshape(1, F)
    Wo = np.ascontiguousarray(np.asarray(Wo, dtype=np.float32))
    bo8 = np.ascontiguousarray(
        np.asarray(bo, dtype=np.float32).reshape(1, O) / np.float32(NCORES)
    )

    nc = _get_nc()
    in_maps = []
    for k in range(NCORES):
        in_maps.append(
            {
                "x": np.ascontiguousarray(x[k * NLOC : (k + 1) * NLOC]),
                "wp": Wp,
                "bp": bp,
                "we": We,
                "be": be,
                "wo": Wo,
                "bo8": bo8,
            }
        )
    res = run_bass_kernel_spmd(nc, in_maps, core_ids=list(range(NCORES)))
    out = np.zeros((C, O), np.float32)
    for r in res.results:
        out = out + r["out"]
    return out[None]  # [1, C, O]



# revision 72
# speedup vs baseline: 1.4126x; 1.2090x over previous
"""DiffPool pooling layer on 8 Trainium2 NeuronCores.

Reference computation (edge_index / batch are unused by the output):
    s      = softmax(x @ Wp + bp, axis=-1)        # [N, C]
    h      = x @ We + be                          # [N, F]
    pooled = s^T @ h                              # [C, F]
    out    = pooled[None] @ Wo + bo               # [1, C, O]

Algebraic restructuring (projection is linear):
    pooled = (s^T x) We + colsum(s) be^T
so per node-shard k each core computes the partials
    G_k  = s_k^T x_k            [C, F]
    cs_k = colsum(s_k)          [C]
    out_k = (G_k We + cs_k be^T) Wo + bo/8
and the host sums the eight [C, O] partials (the unshard step).
No h materialization, no collectives.

Layout: nodes are block-assigned to partitions (partition p holds nodes
p*48..p*48+47 of the first 6144; the 106-node tail is node-major). This
makes the x DMA 8KB-contiguous per partition line (descriptor-cheap).
Any node->partition assignment is valid because the G contraction only
requires s and x to agree on it.

x load (parts="v13", the production path): 12 HWDGE fp32 slice DMAs on the
sync ring (~325 GB/s, near the 358 GB/s/core HBM cap; SWDGE cast-DMA only
manages ~170 GB/s and also poisons the shared SDMA engines, so it is not
used). fp32 slices land in a rotating staging pool; fp32->fp16 casts are
emitted just-in-time inside the compute loop, alternating DVE/ACT, so a
cast waiting on its DMA never head-of-line-blocks an engine FIFO in front
of ready pipeline work.

Compute, per group of B=4 128-node tiles (batched to amortize per-op overhead):
  - PE: 8 transposes -> xT pair (fp16, one PSUM bank)
  - DVE: one bitcast-fp32 copy of the pair -> SBUF
  - PE: one group-wide bias MM (ones^T @ bp replicated B times) + 4
    f-chunk logits MMs per tile (fp16, fp32 PSUM); batching the bias into
    one MM per group instead of one per tile was worth ~5-15us/pass
  - ACT: exp per tile w/ accum_out row sums; DVE: one pair reciprocal +
    per-tile scale -> s (fp16)
  - PE: per tile G += s^T x  [C,512] and cs += s^T 1  [C,1]. cs uses s as
    the stationary (shared with G) and its own PSUM bank: interleaving two
    accumulation groups with ALTERNATING stationaries (the old
    cs = ones^T s form) costs ~1.1us/tile extra on HW; the shared-
    stationary [C,1] form is ~150ns. The [C,1] colsum is transposed to
    [1,C] once, in the projection.
  - G/cs run GSKEW=2 pair-groups behind the softmax chain.
Final (once per core): project the partial in fp32 on PE.
Measured ~61-63us per core-pass on HW vs ~71-73us for the previous
SWDGE-cast-DMA baseline in the same process (device-speed drift between
processes is +-20-30%, so same-process comparison is the only reliable
one).
"""

import numpy as np
from contextlib import ExitStack

N_ALL, F, C, O = 50000, 512, 64, 256
NCORES = 8
NLOC = N_ALL // NCORES          # 6250 nodes per core
P = 128
KC = F // P                     # 4 feature chunks
JROWS = 48                      # node tiles in the main block
NMAIN = P * JROWS               # 6144 nodes in the main block
NTAIL = NLOC - NMAIN            # 106-node tail
NSPLIT = 12                     # main-block DMA slices (4 tiles each)
JS = JROWS // NSPLIT            # tiles per slice

_CACHE = {}


def _main_loop(nc, mybir, x_d, xs_parts, x_tail, xs32pool, x32_tail,
               xtpool, spool, smallp, pxt, plg,
               ident16, ones_row16, ones_col16, wp_h, bp_h, g_ps, cs_ps,
               parts="full32", bp_rep=None):
    """One full pass over this core's node shard, accumulating G / colsum."""
    f32 = mybir.dt.float32
    f16 = mybir.dt.float16
    AF = mybir.ActivationFunctionType

    if parts == "nd_empty":
        # For_i back-edge floor: a trivial body
        zt = smallp.tile([P, 1], f32, tag="zz", name="zz")
        nc.vector.memset(zt[:], 0.0)
        return

    pending_casts = []  # (engine, dst, src, rows) deferred into the loop

    # x DMAs: main block as NSPLIT slices, 16KB contiguous per partition
    xm = x_d[0:NMAIN, :].rearrange("(p j) f -> p j f", p=P)
    if parts == "dma32":
        # ablation: HWDGE fp32 loads (no cast) into fp32 scratch
        for i in range(NSPLIT):
            nc.sync.dma_start(xs_parts[i][:], xm[:, i * JS : (i + 1) * JS, :])
        nc.sync.dma_start(x_tail[0:NTAIL, :], x_d[NMAIN:NLOC, :])
        return
    if parts == "dma32b":
        # ablation: HWDGE fp32 loads split across both HWDGE rings
        for i in range(NSPLIT):
            eng = nc.sync if i % 2 == 0 else nc.scalar
            eng.dma_start(xs_parts[i][:], xm[:, i * JS : (i + 1) * JS, :])
        nc.scalar.dma_start(x_tail[0:NTAIL, :], x_d[NMAIN:NLOC, :])
        return
    if parts in ("full32", "cast32", "v3", "v3c", "v4", "v4c"):
        # v2/v3 load: HWDGE fp32 into rotating staging + on-chip cast to fp16
        def cast_dve(dst, src):
            nc.vector.tensor_copy(dst, src)

        def cast_act(dst, src):
            nc.scalar.activation(dst, src, AF.Copy)

        def cast_gps(dst, src):
            nc.gpsimd.tensor_copy(dst, src)

        if parts in ("full32", "cast32"):
            cast_engs = [cast_gps]
        elif parts in ("v4", "v4c"):
            # spread the cast across the three underused engines
            cast_engs = [cast_dve, cast_act, cast_gps]
        else:
            cast_engs = [cast_dve]
        for i in range(NSPLIT):
            t32 = xs32pool.tile([P, JS, F], f32, tag="xs32", name=f"xs32_{i}")
            nc.sync.dma_start(t32[:], xm[:, i * JS : (i + 1) * JS, :])
            cast_engs[i % len(cast_engs)](xs_parts[i][:], t32[:])
        nc.sync.dma_start(x32_tail[0:NTAIL, :], x_d[NMAIN:NLOC, :])
        cast_engs[0](x_tail[0:NTAIL, :], x32_tail[0:NTAIL, :])
        if parts in ("cast32", "v3c", "v4c"):
            return
    elif parts in ("v5", "v5c"):
        # HWDGE fp32 (+DVE/ACT cast) for the first NHW slices, concurrent
        # SWDGE cast-DMA (hardware fp32->fp16, own queue) for the rest
        NHW = 8
        for i in range(NHW):
            t32 = xs32pool.tile([P, JS, F], f32, tag="xs32", name=f"xs32_{i}")
            nc.sync.dma_start(t32[:], xm[:, i * JS : (i + 1) * JS, :])
            if i % 2 == 0:
                nc.vector.tensor_copy(xs_parts[i][:], t32[:])
            else:
                nc.scalar.activation(xs_parts[i][:], t32[:], AF.Copy)
        for i in range(NHW, NSPLIT):
            nc.gpsimd.dma_start(xs_parts[i][:], xm[:, i * JS : (i + 1) * JS, :])
        nc.gpsimd.dma_start(x_tail[0:NTAIL, :], x_d[NMAIN:NLOC, :])
        if parts == "v5c":
            return
    elif parts in ("v8", "v8d", "v12", "v13", "v14"):
        # HWDGE fp32 for everything; DVE/ACT casts are emitted just-in-time
        # inside the compute loop so they never head-of-line-block the
        # engines' FIFO queues
        for i in range(NSPLIT):
            t32 = xs32pool.tile([P, JS, F], f32, tag="xs32", name=f"xs32_{i}")
            nc.sync.dma_start(t32[:], xm[:, i * JS : (i + 1) * JS, :])
            eng = "dve" if (parts == "v8d" or i % 2 == 0) else "act"  # v8/v12 alternate
            pending_casts.append((eng, xs_parts[i], t32, None))
        nc.sync.dma_start(x32_tail[0:NTAIL, :], x_d[NMAIN:NLOC, :])
        pending_casts.append(("dve", x_tail, x32_tail, NTAIL))
    elif parts in ("v6", "v6c"):
        # DVE/ACT stay out of the load path entirely: SWDGE cast-DMA for the
        # last slices + tail (own queue, hardware cast), HWDGE fp32 + gpsimd
        # cast for the first NHW6 slices. Both DMA paths run concurrently.
        NHW6 = 8
        for i in range(NHW6, NSPLIT):
            nc.gpsimd.dma_start(xs_parts[i][:], xm[:, i * JS : (i + 1) * JS, :])
        nc.gpsimd.dma_start(x_tail[0:NTAIL, :], x_d[NMAIN:NLOC, :])
        for i in range(NHW6):
            t32 = xs32pool.tile([P, JS, F], f32, tag="xs32", name=f"xs32_{i}")
            nc.sync.dma_start(t32[:], xm[:, i * JS : (i + 1) * JS, :])
            nc.gpsimd.tensor_copy(xs_parts[i][:], t32[:])
        if parts == "v6c":
            return
    elif parts in ("full", "dma"):
        nc.gpsimd.dma_start(x_tail[0:NTAIL, :], x_d[NMAIN:NLOC, :])
        for i in range(NSPLIT):
            nc.gpsimd.dma_start(xs_parts[i][:], xm[:, i * JS : (i + 1) * JS, :])
    # nodma / nd_* variants: no x load at all (x memset in _build)

    if parts == "dma":
        return

    # tile list: (x view full-partition, active rows)
    tiles = [(xs_parts[j // JS][:, j % JS, :], P) for j in range(JROWS)]
    tiles.append((x_tail[:, :], NTAIL))
    ntiles = len(tiles)

    if parts in ("v4", "v4n", "v5", "v6", "v8", "v8d", "v12", "v12n", "v13", "v13n", "v14", "v14n"):
        # batch-of-B pipeline: B tiles per PSUM tile / DVE op
        B = 4 if parts in ("v12", "v12n", "v13", "v13n", "v14", "v14n") else 2
        groups = [(B * g, min(B, ntiles - B * g))
                  for g in range((ntiles + B - 1) // B)]
        ngroups = len(groups)
        xt_sbs2 = {}
        s_pairs = {}

        def emit_cast(idx):
            if idx >= len(pending_casts):
                return
            eng, dst, src, rows = pending_casts[idx]
            d = dst[0:rows, :] if rows else dst[:]
            s = src[0:rows, :] if rows else src[:]
            if eng == "dve":
                nc.vector.tensor_copy(d, s)
            else:
                nc.scalar.activation(d, s, AF.Copy)

        def p_transp(g):
            j0, gn = groups[g]
            xt_ps = pxt.tile([P, B, KC, P], f16, tag="xt_ps", name="xt_ps")
            for t in range(gn):
                xv, nt = tiles[j0 + t]
                for k in range(KC):
                    nc.tensor.transpose(
                        xt_ps[:, t, k, 0:nt],
                        xv[0:nt, k * P : (k + 1) * P],
                        ident16[0:nt, 0:nt],
                    )
            xt_sb = xtpool.tile([P, B, KC, P], f16, tag="xt_sb", name="xt_sb")
            nt_last = tiles[j0 + gn - 1][1]
            if gn == B and nt_last == P:
                nc.vector.tensor_copy(
                    xt_sb[:].bitcast(f32), xt_ps[:].bitcast(f32)
                )
            else:
                nc.vector.tensor_copy(
                    xt_sb[:, 0:gn, :, 0:nt_last].bitcast(f32),
                    xt_ps[:, 0:gn, :, 0:nt_last].bitcast(f32),
                )
            xt_sbs2[g] = xt_sb

        def p_logits(g):
            j0, gn = groups[g]
            xt_sb = xt_sbs2.pop(g)
            lg_ps = plg.tile([P, B, C], f32, tag="lg_ps", name="lg_ps")
            if parts in ("v13", "v13n", "v14", "v14n"):
                nt0 = tiles[j0][1]
                nc.tensor.matmul(
                    lg_ps[0:nt0, 0:gn, :], ones_row16[:, 0:nt0],
                    bp_rep[:, 0:gn, :], start=True, stop=False,
                    skip_group_check=True,
                )
                for t in range(gn):
                    nt = tiles[j0 + t][1]
                    for k in range(KC):
                        nc.tensor.matmul(
                            lg_ps[0:nt, t, :], xt_sb[:, t, k, 0:nt],
                            wp_h[:, k, :],
                            start=False, stop=(k == KC - 1),
                            skip_group_check=True,
                        )
                return lg_ps
            for t in range(gn):
                nt = tiles[j0 + t][1]
                nc.tensor.matmul(
                    lg_ps[0:nt, t, :], ones_row16[:, 0:nt], bp_h[:],
                    start=True, stop=False,
                )
                for k in range(KC):
                    nc.tensor.matmul(
                        lg_ps[0:nt, t, :], xt_sb[:, t, k, 0:nt], wp_h[:, k, :],
                        start=False, stop=(k == KC - 1),
                    )
            return lg_ps

        def p_softmax(g, lg_ps):
            j0, gn = groups[g]
            nt_last = tiles[j0 + gn - 1][1]
            sedt = f16 if parts in ("v14", "v14n") else f32
            se = spool.tile([P, B, C], sedt, tag="se", name="se")
            rs = smallp.tile([P, B], f32, tag="rs", name="rs")
            for t in range(gn):
                nt = tiles[j0 + t][1]
                nc.scalar.activation(
                    se[0:nt, t, :], lg_ps[0:nt, t, :], AF.Exp,
                    accum_out=rs[0:nt, t : t + 1],
                )
            ri = smallp.tile([P, B], f32, tag="ri", name="ri")
            if gn == B and nt_last == P:
                nc.vector.reciprocal(ri[:, :], rs[:, :])
            else:
                nc.vector.reciprocal(ri[0:nt_last, 0:gn], rs[0:nt_last, 0:gn])
            s_h = spool.tile([P, B, C], f16, tag="s_h", name="s_h")
            for t in range(gn):
                nt = tiles[j0 + t][1]
                nc.vector.tensor_scalar_mul(
                    s_h[0:nt, t, :], se[0:nt, t, :], ri[0:nt, t : t + 1]
                )
            s_pairs[g] = s_h

        def p_gcs(g, last):
            j0, gn = groups[g]
            s_h = s_pairs.pop(g)
            for t in range(gn):
                xv, nt = tiles[j0 + t]
                is_last = last and t == gn - 1
                nc.tensor.matmul(
                    g_ps[:], s_h[0:nt, t, 0:C], xv[0:nt, :],
                    start=(j0 + t == 0), stop=is_last,
                )
                nc.tensor.matmul(
                    cs_ps[:], s_h[0:nt, t, 0:C], ones_col16[0:nt, :],
                    start=(j0 + t == 0), stop=is_last,
                )

        GSKEW = 2
        next_cast = [0]

        def pace_casts(g):
            # emit casts through (slice consumed by group g) + 1 lookahead
            target = min((B * g) // JS + 2, len(pending_casts))
            while next_cast[0] < target:
                emit_cast(next_cast[0])
                next_cast[0] += 1

        pace_casts(0)
        p_transp(0)
        for g in range(ngroups):
            pace_casts(g + 1)
            if g + 1 < ngroups:
                p_transp(g + 1)
            lg_ps = p_logits(g)
            if g >= GSKEW:
                p_gcs(g - GSKEW, last=False)
            p_softmax(g, lg_ps)
        for g in range(ngroups - GSKEW, ngroups):
            p_gcs(g, last=(g == ngroups - 1))
        return

    # software pipeline so PE never waits on DVE/ACT:
    # at step j, PE runs: transp(j+1) | logits(j) | G/cs(j-SKEW)
    xt_sbs = {}   # j -> xt_sb
    s_views = {}  # j -> s view for G/cs

    nd_g_family = ("nd_g", "nd_gonly", "nd_csonly", "nd_gcs1", "nd_gbatch",
                   "nd_trg")
    cs_col = parts in ("nd_gcs1", "nd_gbatch", "v3", "v3n")
    cs_batch = parts == "nd_gbatch"
    saved_s = []

    def emit_transp(j):
        if (parts in nd_g_family and parts != "nd_trg") or parts == "nd_lgexp":
            return
        xv, nt = tiles[j]
        if parts == "nd_trg":
            nt = P  # tail rows are memset; transpose full width
        xt_ps = pxt.tile([P, KC, P], f16, tag="xt_ps", name="xt_ps")
        for k in range(KC):
            nc.tensor.transpose(
                xt_ps[:, k, 0:nt],
                xv[0:nt, k * P : (k + 1) * P],
                ident16[0:nt, 0:nt],
            )
        xt_sb = xtpool.tile([P, KC, P], f16, tag="xt_sb", name="xt_sb")
        # fp16 pairs copied as fp32 halves the DVE element count
        nc.vector.tensor_copy(
            xt_sb[:, :, 0:nt].bitcast(f32), xt_ps[:, :, 0:nt].bitcast(f32)
        )
        xt_sbs[j] = xt_sb

    def emit_logits(j):
        _, nt = tiles[j]
        if parts in nd_g_family:
            return None
        if parts == "nd_lgexp":
            xt_sb = None
        else:
            xt_sb = xt_sbs.pop(j)
        lg_ps = plg.tile([P, C], f32, tag="lg_ps", name="lg_ps")
        nc.tensor.matmul(
            lg_ps[0:nt, :], ones_row16[:, 0:nt], bp_h[:],
            start=True, stop=False,
        )
        for k in range(KC):
            lhs = ident16[:, 0:nt] if xt_sb is None else xt_sb[:, k, 0:nt]
            nc.tensor.matmul(
                lg_ps[0:nt, :], lhs, wp_h[:, k, :],
                start=False, stop=(k == KC - 1),
            )
        return lg_ps

    def emit_softmax(j, lg_ps):
        _, nt = tiles[j]
        if parts in nd_g_family:
            s_views[j] = ident16[0:nt, 0:C]
            return
        if parts in ("nd_exp", "nd_lgexp"):
            # unnormalized exp straight to fp16 (timing ablation)
            s_h = spool.tile([P, C], f16, tag="s_h", name="s_h")
            rs = smallp.tile([P, 1], f32, tag="rs", name="rs")
            nc.scalar.activation(
                s_h[0:nt, :], lg_ps[0:nt, :], AF.Exp, accum_out=rs[0:nt, :]
            )
            s_views[j] = s_h[0:nt, :]
            return
        se = spool.tile([P, C], f32, tag="se", name="se")
        rs = smallp.tile([P, 1], f32, tag="rs", name="rs")
        nc.scalar.activation(
            se[0:nt, :], lg_ps[0:nt, :], AF.Exp, accum_out=rs[0:nt, :]
        )
        ri = smallp.tile([P, 1], f32, tag="ri", name="ri")
        nc.vector.reciprocal(ri[0:nt, :], rs[0:nt, :])
        s_h = spool.tile([P, C], f16, tag="s_h", name="s_h")
        if parts in ("v3", "v3n"):
            # normalize on ACT (per-partition scale), keeping DVE light
            nc.scalar.activation(
                s_h[0:nt, :], se[0:nt, :], AF.Copy, scale=ri[0:nt, :]
            )
        else:
            nc.vector.tensor_scalar_mul(s_h[0:nt, :], se[0:nt, :], ri[0:nt, :])
        s_views[j] = s_h[0:nt, :]

    def emit_gcs(j, last):
        xv, nt = tiles[j]
        s_view = s_views.pop(j)
        if parts == "nd_trg":
            xt_sb = xt_sbs.pop(j)
            nc.tensor.matmul(
                g_ps[:], ident16[:, 0:C], xt_sb[:, :, :],
                start=(j == 0), stop=last,
            )
            return
        if parts != "nd_csonly":
            nc.tensor.matmul(
                g_ps[:], s_view, xv[0:nt, :],
                start=(j == 0), stop=last,
            )
        if parts in ("nd_gonly",):
            return
        if cs_batch:
            saved_s.append((s_view, nt))
        elif cs_col:
            # cs^T [C,1]: reuse s as the stationary (no weight reload)
            nc.tensor.matmul(
                cs_ps[:], s_view, ones_col16[0:nt, :],
                start=(j == 0), stop=last,
            )
        else:
            nc.tensor.matmul(
                cs_ps[:], ones_col16[0:nt, :], s_view,
                start=(j == 0), stop=last,
            )

    SKEW = 4
    emit_transp(0)
    for j in range(ntiles):
        if j + 1 < ntiles:
            emit_transp(j + 1)
        lg_ps = emit_logits(j)
        if j >= SKEW:
            emit_gcs(j - SKEW, last=False)
        emit_softmax(j, lg_ps)
    for j in range(ntiles - SKEW, ntiles):
        emit_gcs(j, last=(j == ntiles - 1))
    if cs_batch:
        for idx, (sv, nt) in enumerate(saved_s):
            nc.tensor.matmul(
                cs_ps[:], sv, ones_col16[0:nt, :],
                start=(idx == 0), stop=(idx == len(saved_s) - 1),
            )


def _build(bench_reps=None, parts="v13"):
    """Build the bass module. bench_reps: if set, wrap the main node loop
    in a hardware For_i repeating it that many times (timing-only variant:
    x and weights live on device, no input transfer)."""
    import concourse.mybir as mybir
    import concourse.tile as tile
    from concourse import bacc
    from concourse.masks import make_identity

    f32 = mybir.dt.float32
    f16 = mybir.dt.float16

    nc = bacc.Bacc(
        "TRN2", target_bir_lowering=False, debug=False, num_devices=NCORES
    )

    if bench_reps:
        x_d = nc.dram_tensor("xint", [NLOC, F], f32, kind="Internal").ap()
        wp_d = bp_d = we_d = be_d = wo_d = bo_d = None
    else:
        x_d = nc.dram_tensor("x", [NLOC, F], f32, kind="ExternalInput").ap()
        wp_d = nc.dram_tensor("wp", [F, C], f32, kind="ExternalInput").ap()
        bp_d = nc.dram_tensor("bp", [1, C], f32, kind="ExternalInput").ap()
        we_d = nc.dram_tensor("we", [F, F], f32, kind="ExternalInput").ap()
        be_d = nc.dram_tensor("be", [1, F], f32, kind="ExternalInput").ap()
        wo_d = nc.dram_tensor("wo", [F, O], f32, kind="ExternalInput").ap()
        bo_d = nc.dram_tensor("bo8", [1, O], f32, kind="ExternalInput").ap()
    out_d = nc.dram_tensor("out", [C, O], f32, kind="ExternalOutput").ap()

    with tile.TileContext(nc) as tc, ExitStack() as ctx:
        const = ctx.enter_context(tc.tile_pool(name="const", bufs=1))
        accp = ctx.enter_context(tc.tile_pool(name="accp", bufs=1, space="PSUM"))

        ident16 = const.tile([P, P], f16)
        make_identity(nc, ident16[:])
        ident32 = const.tile([C, C], f32)
        make_identity(nc, ident32[:])
        ones_row16 = const.tile([1, P], f16)
        nc.vector.memset(ones_row16[:], 1.0)
        ones_col16 = const.tile([P, 1], f16)
        nc.vector.memset(ones_col16[:], 1.0)
        ones_row32 = const.tile([1, P], f32)
        nc.vector.memset(ones_row32[:], 1.0)

        # resident x (fp16): NSPLIT main slices + node-major tail
        xdt = f32 if parts in ("dma32", "dma32b") else f16
        xs_parts = [
            const.tile([P, JS, F], xdt, name=f"xs{i}") for i in range(NSPLIT)
        ]
        x_tail = const.tile([P, F], xdt, name="x_tail")
        x32_tail = (
            const.tile([P, F], f32, name="x32_tail")
            if parts in ("full32", "cast32", "v3", "v3c", "v4", "v4c", "v8", "v8d", "v12", "v13", "v14")
            else None
        )

        # weights: [F, M] -> [128, KC, M] (partition = f within chunk)
        wp_sb = const.tile([P, KC, C], f32)
        wp_h = const.tile([P, KC, C], f16)
        bp_h = const.tile([1, C], f16)
        we_sb = const.tile([P, KC, F], f32)
        be_sb = const.tile([1, F], f32)
        wo_sb = const.tile([P, KC, O], f32)
        bo_sb = const.tile([1, O], f32)
        if bench_reps:
            for tl in (wp_sb, bp_h, we_sb, be_sb, wo_sb, bo_sb):
                nc.vector.memset(tl[:], 0.0)
        else:
            nc.gpsimd.dma_start(
                wp_sb[:], wp_d.rearrange("(kc p) c -> p kc c", p=P)
            )
            nc.gpsimd.dma_start(bp_h[:], bp_d)  # cast during DMA
            nc.gpsimd.dma_start(
                we_sb[:], we_d.rearrange("(kc p) f -> p kc f", p=P)
            )
            nc.gpsimd.dma_start(be_sb[:], be_d)
            nc.gpsimd.dma_start(
                wo_sb[:], wo_d.rearrange("(kc p) o -> p kc o", p=P)
            )
            nc.gpsimd.dma_start(bo_sb[:], bo_d)
        nc.gpsimd.tensor_copy(wp_h[:], wp_sb[:])
        bp_rep = const.tile([1, 4, C], f16, name="bp_rep")
        for _t in range(4):
            nc.vector.tensor_copy(bp_rep[:, _t, :], bp_h[:])

        # persistent accumulators (one PSUM bank each)
        cs_col_mode = parts in ("nd_gcs1", "nd_gbatch", "v3", "v3n",
                                "v4", "v4n", "v5", "v6", "v8", "v8d", "v12", "v12n", "v13", "v13n", "v14", "v14n")
        g_ps = accp.tile([C, F], f32)
        cs_ps = accp.tile([C, 1] if cs_col_mode else [1, C], f32)
        if parts in ("nd_gonly", "nd_trg"):
            nc.vector.memset(cs_ps[:], 0.0)
        if parts == "nd_csonly":
            nc.vector.memset(g_ps[:], 0.0)

        if bench_reps:
            # zero-fill internal x so the compute sees finite data
            zt = const.tile([P, JS, F], f32, name="zt")
            nc.vector.memset(zt[:], 0.0)
            xm = x_d[0:NMAIN, :].rearrange("(p j) f -> p j f", p=P)
            for i in range(NSPLIT):
                nc.sync.dma_start(xm[:, i * JS : (i + 1) * JS, :], zt[:])
            nc.sync.dma_start(x_d[NMAIN:NLOC, :], zt[0:NTAIL, 0, :])
            if parts in ("nodma", "v3n", "v4n", "v12n", "v13n", "v14n") or parts.startswith("nd_"):
                for i in range(NSPLIT):
                    nc.vector.memset(xs_parts[i][:], 0.0)
                nc.vector.memset(x_tail[:], 0.0)

        with ExitStack() as lctx:
            xtpool = lctx.enter_context(tc.tile_pool(name="xtpool", bufs=4))
            spool = lctx.enter_context(tc.tile_pool(name="spool", bufs=8))
            smallp = lctx.enter_context(tc.tile_pool(name="smallp", bufs=8))
            xs32pool = lctx.enter_context(tc.tile_pool(name="xs32p", bufs=6))
            psb = 2 if parts in ("v12", "v12n", "v13", "v13n", "v14", "v14n") else 3
            pxt = lctx.enter_context(
                tc.tile_pool(name="pxt", bufs=psb, space="PSUM")
            )
            plg = lctx.enter_context(
                tc.tile_pool(name="plg", bufs=psb, space="PSUM")
            )

            rep_ctx = (
                tc.For_i(0, bench_reps, 1) if bench_reps else ExitStack()
            )
            with rep_ctx:
                _main_loop(
                    nc, mybir, x_d, xs_parts, x_tail, xs32pool, x32_tail,
                    xtpool, spool, smallp, pxt, plg,
                    ident16, ones_row16, ones_col16, wp_h, bp_h,
                    g_ps, cs_ps, parts=parts, bp_rep=bp_rep,
                )

        if parts in ("dma", "dma32", "dma32b", "cast32", "v3c", "v4c",
                     "v5c", "v6c", "nd_empty"):
            with ExitStack() as fctx:
                fin0 = fctx.enter_context(tc.tile_pool(name="fin0", bufs=1))
                dummy = fin0.tile([C, O], f32, name="dummy")
                nc.vector.memset(dummy[:], 0.0)
                nc.sync.dma_start(out_d, dummy[:])
        elif True:
            # ---- final projection of the per-core partial (fp32) ----
            with ExitStack() as fctx:
                fin = fctx.enter_context(tc.tile_pool(name="fin", bufs=1))
                pfin = fctx.enter_context(
                    tc.tile_pool(name="pfin", bufs=1, space="PSUM")
                )

                g_sb = fin.tile([C, F], f32)
                nc.vector.tensor_copy(g_sb[:], g_ps[:])
                if cs_col_mode:
                    csc_sb = fin.tile([C, 1], f32)
                    nc.vector.tensor_copy(csc_sb[:], cs_ps[:])
                    csT_ps = pfin.tile([1, C], f32, name="csT_ps")
                    nc.tensor.transpose(
                        csT_ps[:], csc_sb[:], ident32[0:C, 0:C]
                    )
                    cs_sb = fin.tile([1, C], f32)
                    nc.vector.tensor_copy(cs_sb[:], csT_ps[:])
                else:
                    cs_sb = fin.tile([1, C], f32)
                    nc.vector.tensor_copy(cs_sb[:], cs_ps[:])

                # G^T chunks [128, C] so fin lands on partitions
                gt_ps = pfin.tile([P, KC, C], f32, name="gt_ps")
                for k in range(KC):
                    nc.tensor.transpose(
                        gt_ps[:, k, :], g_sb[:, k * P : (k + 1) * P], ident32[:]
                    )
                gt_sb = fin.tile([P, KC, C], f32)
                nc.vector.tensor_copy(gt_sb[:], gt_ps[:])

                # pooledT[fo, c] = sum_fin We[fin, fo] G^T[fin, c] + be[fo] cs[c]
                pt_ps = pfin.tile([P, KC, C], f32, name="pt_ps")
                for j in range(KC):
                    nc.tensor.matmul(
                        pt_ps[:, j, :],
                        be_sb[:, j * P : (j + 1) * P],
                        cs_sb[:],
                        start=True,
                        stop=False,
                    )
                    for k in range(KC):
                        nc.tensor.matmul(
                            pt_ps[:, j, :],
                            we_sb[:, k, j * P : (j + 1) * P],
                            gt_sb[:, k, :],
                            start=False,
                            stop=(k == KC - 1),
                        )
                pt_sb = fin.tile([P, KC, C], f32)
                nc.vector.tensor_copy(pt_sb[:], pt_ps[:])

                # out[c, o] = sum_fo pooledT[fo, c] Wo[fo, o] + bo/8
                out_ps = pfin.tile([C, O], f32, name="out_ps")
                nc.tensor.matmul(
                    out_ps[:], ones_row32[:, 0:C], bo_sb[:],
                    start=True, stop=False,
                )
                for j in range(KC):
                    nc.tensor.matmul(
                        out_ps[:], pt_sb[:, j, :], wo_sb[:, j, :],
                        start=False, stop=(j == KC - 1),
                    )
                out_sb = fin.tile([C, O], f32)
                nc.vector.tensor_copy(out_sb[:], out_ps[:])
                nc.sync.dma_start(out_d, out_sb[:])

    nc.compile()
    return nc


def _get_nc(bench_reps=None, parts="v13"):
    key = ("nc", bench_reps, parts)
    if key not in _CACHE:
        _CACHE[key] = _build(bench_reps, parts)
    return _CACHE[key]


def kernel(x, edge_index=None, batch=None, Wp=None, bp=None, We=None,
           be=None, Wo=None, bo=None, **_unused):
    from concourse.bass_utils import run_bass_kernel_spmd

    x = np.ascontiguousarray(np.asarray(x, dtype=np.float32))
    Wp = np.ascontiguousarray(np.asarray(Wp, dtype=np.float32))
    bp = np.ascontiguousarray(np.asarray(bp, dtype=np.float32)).reshape(1, C)
    We = np.ascontiguousarray(np.asarray(We, dtype=np.float32))
    be = np.ascontiguousarray(np.asarray(be, dtype=np.float32)).reshape(1, F)
    Wo = np.ascontiguousarray(np.asarray(Wo, dtype=np.float32))
    bo8 = np.ascontiguousarray(
        np.asarray(bo, dtype=np.float32).reshape(1, O) / np.float32(NCORES)
    )

    nc = _get_nc()
    in_maps = []
    for k in range(NCORES):
        in_maps.append(
            {
                "x": np.ascontiguousarray(x[k * NLOC : (k + 1) * NLOC]),
                "wp": Wp,
                "bp": bp,
                "we": We,
                "be": be,
                "wo": Wo,
                "bo8": bo8,
            }
        )
    res = run_bass_kernel_spmd(nc, in_maps, core_ids=list(range(NCORES)))
    out = np.zeros((C, O), np.float32)
    for r in res.results:
        out = out + r["out"]
    return out[None]  # [1, C, O]

